# revision 2
# baseline (speedup 1.0000x reference)
"""Trainium2 Bass kernel for the NTN problem.

out[b,k,q,a] = sigmoid( q[b,q,:] @ w[k] @ da[b,a,:]
                        + Vq[k]@q[b,q,:] + Vd[k]@da[b,a,:] + b[k] )

B=64, K=16, Q=A=D=256.  Sharding: data-parallel over batch B across the
8 NeuronCores (8 batches per core); w/V/b replicated.

Per core, per (k, batch-pair):
  MM1 (TensorE, float32r): tmp[e, q|q'] = sum_d w[k,d,e]^T qT[d, q|q']   (N=512)
  DVE: tmp PSUM->SBUF with per-partition bias +Vd[k,e]  (folds Vd@da term)
  MM2 (TensorE, float32r): out[q, a] = sum_e tmp[e,q]^T daT[e, a]
  ScalarE: sigmoid(psum + bias mq[b,k,q]) where mq = Vq@q + b  (host-prepped)
"""

import os
import sys
import types
from contextlib import ExitStack

if "/opt/trn_rl_repo" not in sys.path:
    sys.path.insert(0, "/opt/trn_rl_repo")

import numpy as np

import concourse.bass as bass
import concourse.tile as tile
from concourse import bacc, bass_utils, mybir

F32 = mybir.dt.float32
F32R = mybir.dt.float32r
SIG = mybir.ActivationFunctionType.Sigmoid

NCORES = 8
B, Q, A, D, K = 64, 256, 256, 256, 16
E = D
BL = B // NCORES


def _install_profshim():
    """Provide antenv.axon_hooks so trace=True works under axon (best-effort)."""
    try:
        if "antenv.axon_hooks" in sys.modules:
            return True
        import antenv

        mod = types.ModuleType("antenv.axon_hooks")
        holder = {}
        mod.set_axon_ntff_profile_hook = lambda h: holder.__setitem__("h", h)
        mod.get_axon_ntff_profile_hook = lambda: holder.get("h")
        sys.modules["antenv.axon_hooks"] = mod
        antenv.axon_hooks = mod
        from trn_agent_boot.trn_boot import _ntff_profile_via_ctypes

        hook = _ntff_profile_via_ctypes("/opt/axon/libaxon_pjrt.so")
        if hook is None:
            return False
        mod.set_axon_ntff_profile_hook(hook)
        return True
    except Exception:
        return False


def _build_ntn(tc: tile.TileContext, ctx: ExitStack, aps: dict):
    nc = tc.nc
    DC, ET, QT = D // 128, E // 128, Q // 128
    qt, dat, w, vdt, mq, out = (aps[n] for n in ("qt", "dat", "w", "vdt", "mq", "out"))

    w_pool = ctx.enter_context(tc.tile_pool(name="w", bufs=1))
    const_pool = ctx.enter_context(tc.tile_pool(name="const", bufs=1))
    q_pool = ctx.enter_context(tc.tile_pool(name="q", bufs=3))
    da_pool = ctx.enter_context(tc.tile_pool(name="da", bufs=3))
    tmp_pool = ctx.enter_context(tc.tile_pool(name="tmp", bufs=3))
    out_pool = ctx.enter_context(tc.tile_pool(name="out", bufs=8))
    ptmp_pool = ctx.enter_context(tc.tile_pool(name="ptmp", bufs=3, space="PSUM"))
    pout_pool = ctx.enter_context(tc.tile_pool(name="pout", bufs=4, space="PSUM"))

    w_sb = w_pool.tile([128, K, DC, E], F32R)
    nc.sync.dma_start(w_sb[:], w.rearrange("k (dc p) e -> p k dc e", p=128))
    vdt_sb = const_pool.tile([128, ET, K], F32)
    nc.sync.dma_start(vdt_sb[:], vdt.rearrange("(et p) k -> p et k", p=128))
    mq_sb = const_pool.tile([128, QT, BL, K], F32)
    nc.sync.dma_start(mq_sb[:], mq.rearrange("t p b k -> p t b k"))

    for bp in range(BL // 2):
        b0, b1 = 2 * bp, 2 * bp + 1
        q2 = q_pool.tile([128, DC, 2 * Q], F32R)
        nc.sync.dma_start(q2[:, :, 0:Q], qt[b0].rearrange("(dc p) q -> p dc q", p=128))
        nc.sync.dma_start(q2[:, :, Q:2 * Q], qt[b1].rearrange("(dc p) q -> p dc q", p=128))
        da2 = da_pool.tile([128, ET, 2 * A], F32R)
        nc.sync.dma_start(da2[:, :, 0:A], dat[b0].rearrange("(et p) a -> p et a", p=128))
        nc.sync.dma_start(da2[:, :, A:2 * A], dat[b1].rearrange("(et p) a -> p et a", p=128))

        for k in range(K):
            ptmps = []
            for et in range(ET):
                pt = ptmp_pool.tile([128, 2 * Q], F32)
                for dc in range(DC):
                    nc.tensor.matmul(
                        pt[:],
                        lhsT=w_sb[:, k, dc, et * 128:(et + 1) * 128],
                        rhs=q2[:, dc, :],
                        start=(dc == 0),
                        stop=(dc == DC - 1),
                    )
                ptmps.append(pt)
            tmp = tmp_pool.tile([128, ET, 2 * Q], F32R)
            for et in range(ET):
                nc.vector.tensor_scalar_add(
                    tmp[:, et, :], ptmps[et][:], vdt_sb[:, et, k:k + 1]
                )
            for h, b in ((0, b0), (1, b1)):
                for qt_i in range(QT):
                    po = pout_pool.tile([128, A], F32)
                    for et in range(ET):
                        nc.tensor.matmul(
                            po[:],
                            lhsT=tmp[:, et, h * Q + qt_i * 128: h * Q + (qt_i + 1) * 128],
                            rhs=da2[:, et, h * A:(h + 1) * A],
                            start=(et == 0),
                            stop=(et == ET - 1),
                        )
                    o = out_pool.tile([128, A], F32)
                    nc.scalar.activation(
                        o[:], po[:], SIG, bias=mq_sb[:, qt_i, b, k:k + 1]
                    )
                    nc.sync.dma_start(out[b, k, qt_i * 128:(qt_i + 1) * 128, :], o[:])


_COMPILED = None


def _get_compiled():
    global _COMPILED
    if _COMPILED is not None:
        return _COMPILED
    nc = bacc.Bacc("TRN2", target_bir_lowering=False, debug=False, num_devices=NCORES)
    aps = {
        "qt": nc.dram_tensor("qt", [BL, D, Q], F32R, kind="ExternalInput").ap(),
        "dat": nc.dram_tensor("dat", [BL, E, A], F32R, kind="ExternalInput").ap(),
        "w": nc.dram_tensor("w", [K, D, E], F32R, kind="ExternalInput").ap(),
        "vdt": nc.dram_tensor("vdt", [E, K], F32, kind="ExternalInput").ap(),
        "mq": nc.dram_tensor("mq", [Q // 128, 128, BL, K], F32, kind="ExternalInput").ap(),
        "out": nc.dram_tensor("out", [BL, K, Q, A], F32, kind="ExternalOutput").ap(),
    }
    with tile.TileContext(nc) as tc:
        with ExitStack() as ctx:
            _build_ntn(tc, ctx, aps)
    nc.compile()
    _COMPILED = nc
    return nc


def _tf32_rne(x):
    """Round fp32 array to TF32 (10-bit mantissa), round-to-nearest-even."""
    u = x.view(np.uint32)
    r = (u + 0xFFF + ((u >> 13) & 1)) & np.uint32(0xFFFFE000)
    return r.view(np.float32)


def kernel(batch_q_em, batch_da_em, w, V, b):
    q = np.ascontiguousarray(np.asarray(batch_q_em, dtype=np.float32))
    da = np.ascontiguousarray(np.asarray(batch_da_em, dtype=np.float32))
    w = np.ascontiguousarray(np.asarray(w, dtype=np.float32))
    V = np.ascontiguousarray(np.asarray(V, dtype=np.float32))
    b = np.asarray(b, dtype=np.float32).reshape(-1)

    qt = _tf32_rne(np.ascontiguousarray(q.transpose(0, 2, 1)))      # [B, D, Q]
    dat = _tf32_rne(np.ascontiguousarray(da.transpose(0, 2, 1)))    # [B, E, A]
    vdt = np.ascontiguousarray(V[:, D:].T)               # [E, K]
    # mq[b,q,k] = q[b] @ Vq^T + bias
    mqT = q @ V[:, :D].T + b[None, None, :]              # [B, Q, K]

    w = _tf32_rne(w)
    nc = _get_compiled()
    in_maps = []
    for c in range(NCORES):
        s = slice(c * BL, (c + 1) * BL)
        mq_shard = np.ascontiguousarray(
            mqT[s].reshape(BL, Q // 128, 128, K).transpose(1, 2, 0, 3)
        )  # [QT, 128, BL, K]
        in_maps.append({
            "qt": np.ascontiguousarray(qt[s]),
            "dat": np.ascontiguousarray(dat[s]),
            "w": w,
            "vdt": vdt,
            "mq": mq_shard,
        })

    trace = bool(int(os.environ.get("NTN_TRACE", "0"))) and _install_profshim()
    res = bass_utils.run_bass_kernel_spmd(
        nc, in_maps, core_ids=list(range(NCORES)), trace=trace
    )
    if trace and res.exec_time_ns is not None:
        print(f"HW exec time: {res.exec_time_ns} ns")
    out = np.concatenate([r["out"] for r in res.results], axis=0)
    return out


# revision 5
# speedup vs baseline: 1.2531x; 1.2531x over previous
"""Trainium2 Bass kernel for the NTN problem.

out[b,k,q,a] = sigmoid( q[b,q,:] @ w[k] @ da[b,a,:]
                        + Vq[k]@q[b,q,:] + Vd[k]@da[b,a,:] + b[k] )

B=64, K=16, Q=A=D=256.  Sharding: data-parallel over batch B across the
8 NeuronCores (8 batches per core); w/V/b replicated.

Per core, per (k, batch-pair):
  MM1 (TensorE, fp16): tmp[e, q|q'] = sum_d w[k,d,e]^T qT[d, q|q']   (N=512)
  DVE: tmp PSUM->SBUF (fp16) with per-partition bias +Vd[k,e] (folds Vd@da)
  MM2 (TensorE, fp16): out[q, a] = sum_e tmp[e,q]^T daT[e, a]
  ScalarE: sigmoid(psum + bias mq[b,k,q]) where mq = Vq@q + b (host-prepped),
           written into a per-(b,qtile) collect tile covering all 16 k
  One 2 MB DMA per (b, qtile) collect tile -> 16 output stores total.
"""

import os
import sys
import types
from contextlib import ExitStack

if "/opt/trn_rl_repo" not in sys.path:
    sys.path.insert(0, "/opt/trn_rl_repo")

import numpy as np

import concourse.bass as bass
import concourse.tile as tile
from concourse import bacc, bass_utils, mybir

F32 = mybir.dt.float32
F16 = mybir.dt.float16
SIG = mybir.ActivationFunctionType.Sigmoid

NCORES = 8
B, Q, A, D, K = 64, 256, 256, 256, 16
E = D
BL = B // NCORES


def _install_profshim():
    """Provide antenv.axon_hooks so trace=True works under axon (best-effort)."""
    try:
        if "antenv.axon_hooks" in sys.modules:
            return True
        import antenv

        mod = types.ModuleType("antenv.axon_hooks")
        holder = {}
        mod.set_axon_ntff_profile_hook = lambda h: holder.__setitem__("h", h)
        mod.get_axon_ntff_profile_hook = lambda: holder.get("h")
        sys.modules["antenv.axon_hooks"] = mod
        antenv.axon_hooks = mod
        from trn_agent_boot.trn_boot import _ntff_profile_via_ctypes

        hook = _ntff_profile_via_ctypes("/opt/axon/libaxon_pjrt.so")
        if hook is None:
            return False
        mod.set_axon_ntff_profile_hook(hook)
        return True
    except Exception:
        return False


def _build_ntn(tc: tile.TileContext, ctx: ExitStack, aps: dict):
    nc = tc.nc
    DC, ET, QT = D // 128, E // 128, Q // 128
    qt, dat, w, vdt, mq, out = (aps[n] for n in ("qt", "dat", "w", "vdt", "mq", "out"))

    w_pool = ctx.enter_context(tc.tile_pool(name="w", bufs=1))
    const_pool = ctx.enter_context(tc.tile_pool(name="const", bufs=1))
    q_pool = ctx.enter_context(tc.tile_pool(name="q", bufs=3))
    da_pool = ctx.enter_context(tc.tile_pool(name="da", bufs=3))
    tmp_pool = ctx.enter_context(tc.tile_pool(name="tmp", bufs=3))
    out_pool = ctx.enter_context(tc.tile_pool(name="out", bufs=6))
    ptmp_pool = ctx.enter_context(tc.tile_pool(name="ptmp", bufs=3, space="PSUM"))
    pout_pool = ctx.enter_context(tc.tile_pool(name="pout", bufs=5, space="PSUM"))

    w_sb = w_pool.tile([128, K, DC, E], F16)
    nc.gpsimd.dma_start(w_sb[:], w.rearrange("k (dc p) e -> p k dc e", p=128))
    vdt_sb = const_pool.tile([128, ET, K], F32)
    nc.sync.dma_start(vdt_sb[:], vdt.rearrange("(et p) k -> p et k", p=128))
    mq_sb = const_pool.tile([128, QT, BL, K], F32)
    nc.gpsimd.dma_start(mq_sb[:], mq.rearrange("t p b k -> p t b k"))

    for bp in range(BL // 2):
        b0, b1 = 2 * bp, 2 * bp + 1
        q2 = q_pool.tile([128, DC, 2 * Q], F16)
        nc.gpsimd.dma_start(q2[:, :, 0:Q], qt[b0].rearrange("(dc p) q -> p dc q", p=128))
        nc.gpsimd.dma_start(q2[:, :, Q:2 * Q], qt[b1].rearrange("(dc p) q -> p dc q", p=128))
        da2 = da_pool.tile([128, ET, 2 * A], F16)
        nc.gpsimd.dma_start(da2[:, :, 0:A], dat[b0].rearrange("(et p) a -> p et a", p=128))
        nc.gpsimd.dma_start(da2[:, :, A:2 * A], dat[b1].rearrange("(et p) a -> p et a", p=128))

        # per-(b, qtile) collect tiles spanning all K slices
        coll = {(h, qt_i): out_pool.tile([128, K, A], F32, name="coll", tag="coll")
                for h in (0, 1) for qt_i in range(QT)}

        for k in range(K):
            ptmps = []
            for et in range(ET):
                pt = ptmp_pool.tile([128, 2 * Q], F32)
                for dc in range(DC):
                    nc.tensor.matmul(
                        pt[:],
                        lhsT=w_sb[:, k, dc, et * 128:(et + 1) * 128],
                        rhs=q2[:, dc, :],
                        start=(dc == 0),
                        stop=(dc == DC - 1),
                    )
                ptmps.append(pt)
            tmp = tmp_pool.tile([128, ET, 2 * Q], F16)
            for et in range(ET):
                nc.vector.tensor_scalar_add(
                    tmp[:, et, :], ptmps[et][:], vdt_sb[:, et, k:k + 1]
                )
            for h, b in ((0, b0), (1, b1)):
                for qt_i in range(QT):
                    po = pout_pool.tile([128, A], F32)
                    for et in range(ET):
                        nc.tensor.matmul(
                            po[:],
                            lhsT=tmp[:, et, h * Q + qt_i * 128: h * Q + (qt_i + 1) * 128],
                            rhs=da2[:, et, h * A:(h + 1) * A],
                            start=(et == 0),
                            stop=(et == ET - 1),
                        )
                    nc.scalar.activation(
                        coll[(h, qt_i)][:, k, :], po[:], SIG,
                        bias=mq_sb[:, qt_i, b, k:k + 1],
                    )
        for h, b in ((0, b0), (1, b1)):
            for qt_i in range(QT):
                nc.sync.dma_start(
                    out[b, :, qt_i * 128:(qt_i + 1) * 128, :].rearrange("k p a -> p k a"),
                    coll[(h, qt_i)][:],
                )


_COMPILED = None


def _get_compiled():
    global _COMPILED
    if _COMPILED is not None:
        return _COMPILED
    nc = bacc.Bacc("TRN2", target_bir_lowering=False, debug=False, num_devices=NCORES)
    aps = {
        "qt": nc.dram_tensor("qt", [BL, D, Q], F16, kind="ExternalInput").ap(),
        "dat": nc.dram_tensor("dat", [BL, E, A], F16, kind="ExternalInput").ap(),
        "w": nc.dram_tensor("w", [K, D, E], F16, kind="ExternalInput").ap(),
        "vdt": nc.dram_tensor("vdt", [E, K], F32, kind="ExternalInput").ap(),
        "mq": nc.dram_tensor("mq", [Q // 128, 128, BL, K], F32, kind="ExternalInput").ap(),
        "out": nc.dram_tensor("out", [BL, K, Q, A], F32, kind="ExternalOutput").ap(),
    }
    with tile.TileContext(nc) as tc:
        with ExitStack() as ctx:
            _build_ntn(tc, ctx, aps)
    nc.compile()
    _COMPILED = nc
    return nc


def kernel(batch_q_em, batch_da_em, w, V, b):
    q = np.ascontiguousarray(np.asarray(batch_q_em, dtype=np.float32))
    da = np.ascontiguousarray(np.asarray(batch_da_em, dtype=np.float32))
    w = np.ascontiguousarray(np.asarray(w, dtype=np.float32))
    V = np.ascontiguousarray(np.asarray(V, dtype=np.float32))
    b = np.asarray(b, dtype=np.float32).reshape(-1)

    qt = np.ascontiguousarray(q.transpose(0, 2, 1)).astype(np.float16)   # [B, D, Q]
    dat = np.ascontiguousarray(da.transpose(0, 2, 1)).astype(np.float16)  # [B, E, A]
    w16 = w.astype(np.float16)
    vdt = np.ascontiguousarray(V[:, D:].T)               # [E, K]
    # mq[b,q,k] = q[b] @ Vq^T + bias
    mqT = q @ V[:, :D].T + b[None, None, :]              # [B, Q, K]

    nc = _get_compiled()
    in_maps = []
    for c in range(NCORES):
        s = slice(c * BL, (c + 1) * BL)
        mq_shard = np.ascontiguousarray(
            mqT[s].reshape(BL, Q // 128, 128, K).transpose(1, 2, 0, 3)
        )  # [QT, 128, BL, K]
        in_maps.append({
            "qt": np.ascontiguousarray(qt[s]),
            "dat": np.ascontiguousarray(dat[s]),
            "w": w16,
            "vdt": vdt,
            "mq": mq_shard,
        })

    trace = bool(int(os.environ.get("NTN_TRACE", "0"))) and _install_profshim()
    res = bass_utils.run_bass_kernel_spmd(
        nc, in_maps, core_ids=list(range(NCORES)), trace=trace
    )
    if trace and res.exec_time_ns is not None:
        print(f"HW exec time: {res.exec_time_ns} ns")
    out = np.concatenate([r["out"] for r in res.results], axis=0)
    return out


# revision 6
# speedup vs baseline: 1.4465x; 1.1543x over previous
"""Trainium2 Bass kernel for the NTN problem.

out[b,k,q,a] = sigmoid( q[b,q,:] @ w[k] @ da[b,a,:]
                        + Vq[k]@q[b,q,:] + Vd[k]@da[b,a,:] + b[k] )

B=64, K=16, Q=A=D=256.  Sharding: data-parallel over batch B across the
8 NeuronCores (8 batches per core); w/V/b replicated.

Per core, per (k, batch-pair):
  MM1 (TensorE, fp16): tmp[e, q|q'] = sum_d w[k,d,e]^T qT[d, q|q']   (N=512)
  DVE: tmp PSUM->SBUF (fp16) with per-partition bias +Vd[k,e] (folds Vd@da)
  MM2 (TensorE, fp16): out[q, a] = sum_e tmp[e,q]^T daT[e, a]
  ScalarE: sigmoid(psum + bias mq[b,k,q]) where mq = Vq@q + b (host-prepped),
           written into a per-(b,qtile) collect tile covering all 16 k
  One 2 MB DMA per (b, qtile) collect tile -> 16 output stores total.
"""

import os
import sys
import types
from contextlib import ExitStack

if "/opt/trn_rl_repo" not in sys.path:
    sys.path.insert(0, "/opt/trn_rl_repo")

import numpy as np

import concourse.bass as bass
import concourse.tile as tile
from concourse import bacc, bass_utils, mybir

F32 = mybir.dt.float32
F16 = mybir.dt.float16
SIG = mybir.ActivationFunctionType.Sigmoid

NCORES = 8
B, Q, A, D, K = 64, 256, 256, 256, 16
E = D
BL = B // NCORES


def _install_profshim():
    """Provide antenv.axon_hooks so trace=True works under axon (best-effort)."""
    try:
        if "antenv.axon_hooks" in sys.modules:
            return True
        import antenv

        mod = types.ModuleType("antenv.axon_hooks")
        holder = {}
        mod.set_axon_ntff_profile_hook = lambda h: holder.__setitem__("h", h)
        mod.get_axon_ntff_profile_hook = lambda: holder.get("h")
        sys.modules["antenv.axon_hooks"] = mod
        antenv.axon_hooks = mod
        from trn_agent_boot.trn_boot import _ntff_profile_via_ctypes

        hook = _ntff_profile_via_ctypes("/opt/axon/libaxon_pjrt.so")
        if hook is None:
            return False
        mod.set_axon_ntff_profile_hook(hook)
        return True
    except Exception:
        return False


def _build_ntn(tc: tile.TileContext, ctx: ExitStack, aps: dict):
    nc = tc.nc
    DC, ET, QT = D // 128, E // 128, Q // 128
    qt, dat, w, vdt, mq, out = (aps[n] for n in ("qt", "dat", "w", "vdt", "mq", "out"))

    w_pool = ctx.enter_context(tc.tile_pool(name="w", bufs=1))
    const_pool = ctx.enter_context(tc.tile_pool(name="const", bufs=1))
    q_pool = ctx.enter_context(tc.tile_pool(name="q", bufs=3))
    da_pool = ctx.enter_context(tc.tile_pool(name="da", bufs=3))
    tmp_pool = ctx.enter_context(tc.tile_pool(name="tmp", bufs=4))
    out_pool = ctx.enter_context(tc.tile_pool(name="out", bufs=12))
    ptmp_pool = ctx.enter_context(tc.tile_pool(name="ptmp", bufs=3, space="PSUM"))
    pout_pool = ctx.enter_context(tc.tile_pool(name="pout", bufs=5, space="PSUM"))

    w_sb = []
    for k in range(K):
        wk = w_pool.tile([128, DC, E], F16, name=f"wk{k}", tag=f"wk{k}")
        nc.sync.dma_start(wk[:], w[k].rearrange("(dc p) e -> p dc e", p=128))
        w_sb.append(wk)
    vdt_sb = const_pool.tile([128, ET, K], F32)
    nc.sync.dma_start(vdt_sb[:], vdt.rearrange("(et p) k -> p et k", p=128))
    mq_sb = const_pool.tile([128, QT, BL, K], F32)
    nc.gpsimd.dma_start(mq_sb[:], mq.rearrange("t p b k -> p t b k"))

    for bp in range(BL // 2):
        b0, b1 = 2 * bp, 2 * bp + 1
        q2 = q_pool.tile([128, DC, 2 * Q], F16)
        nc.gpsimd.dma_start(q2[:, :, 0:Q], qt[b0].rearrange("(dc p) q -> p dc q", p=128))
        nc.gpsimd.dma_start(q2[:, :, Q:2 * Q], qt[b1].rearrange("(dc p) q -> p dc q", p=128))
        da2 = da_pool.tile([128, ET, 2 * A], F16)
        nc.gpsimd.dma_start(da2[:, :, 0:A], dat[b0].rearrange("(et p) a -> p et a", p=128))
        nc.gpsimd.dma_start(da2[:, :, A:2 * A], dat[b1].rearrange("(et p) a -> p et a", p=128))

        # per-(b, qtile, k-half) collect tiles
        KH = K // 2
        coll = {(h, qt_i, kh): out_pool.tile([128, KH, A], F32, name="coll", tag="coll")
                for h in (0, 1) for qt_i in range(QT) for kh in (0, 1)}

        for k in range(K):
            ptmps = []
            for et in range(ET):
                pt = ptmp_pool.tile([128, 2 * Q], F32)
                for dc in range(DC):
                    nc.tensor.matmul(
                        pt[:],
                        lhsT=w_sb[k][:, dc, et * 128:(et + 1) * 128],
                        rhs=q2[:, dc, :],
                        start=(dc == 0),
                        stop=(dc == DC - 1),
                    )
                ptmps.append(pt)
            tmp = tmp_pool.tile([128, ET, 2 * Q], F16)
            for et in range(ET):
                nc.vector.tensor_scalar_add(
                    tmp[:, et, :], ptmps[et][:], vdt_sb[:, et, k:k + 1]
                )
            for h, b in ((0, b0), (1, b1)):
                for qt_i in range(QT):
                    po = pout_pool.tile([128, A], F32)
                    for et in range(ET):
                        nc.tensor.matmul(
                            po[:],
                            lhsT=tmp[:, et, h * Q + qt_i * 128: h * Q + (qt_i + 1) * 128],
                            rhs=da2[:, et, h * A:(h + 1) * A],
                            start=(et == 0),
                            stop=(et == ET - 1),
                        )
                    nc.scalar.activation(
                        coll[(h, qt_i, k // KH)][:, k % KH, :], po[:], SIG,
                        bias=mq_sb[:, qt_i, b, k:k + 1],
                    )
            if (k + 1) % KH == 0:
                kh = k // KH
                for h, b in ((0, b0), (1, b1)):
                    for qt_i in range(QT):
                        nc.sync.dma_start(
                            out[b, kh * KH:(kh + 1) * KH,
                                qt_i * 128:(qt_i + 1) * 128, :].rearrange("k p a -> p k a"),
                            coll[(h, qt_i, kh)][:],
                        )


_COMPILED = None


def _get_compiled():
    global _COMPILED
    if _COMPILED is not None:
        return _COMPILED
    nc = bacc.Bacc("TRN2", target_bir_lowering=False, debug=False, num_devices=NCORES)
    aps = {
        "qt": nc.dram_tensor("qt", [BL, D, Q], F16, kind="ExternalInput").ap(),
        "dat": nc.dram_tensor("dat", [BL, E, A], F16, kind="ExternalInput").ap(),
        "w": nc.dram_tensor("w", [K, D, E], F16, kind="ExternalInput").ap(),
        "vdt": nc.dram_tensor("vdt", [E, K], F32, kind="ExternalInput").ap(),
        "mq": nc.dram_tensor("mq", [Q // 128, 128, BL, K], F32, kind="ExternalInput").ap(),
        "out": nc.dram_tensor("out", [BL, K, Q, A], F32, kind="ExternalOutput").ap(),
    }
    with tile.TileContext(nc) as tc:
        with ExitStack() as ctx:
            _build_ntn(tc, ctx, aps)
    nc.compile()
    _COMPILED = nc
    return nc


def kernel(batch_q_em, batch_da_em, w, V, b):
    q = np.ascontiguousarray(np.asarray(batch_q_em, dtype=np.float32))
    da = np.ascontiguousarray(np.asarray(batch_da_em, dtype=np.float32))
    w = np.ascontiguousarray(np.asarray(w, dtype=np.float32))
    V = np.ascontiguousarray(np.asarray(V, dtype=np.float32))
    b = np.asarray(b, dtype=np.float32).reshape(-1)

    qt = np.ascontiguousarray(q.transpose(0, 2, 1)).astype(np.float16)   # [B, D, Q]
    dat = np.ascontiguousarray(da.transpose(0, 2, 1)).astype(np.float16)  # [B, E, A]
    w16 = w.astype(np.float16)
    vdt = np.ascontiguousarray(V[:, D:].T)               # [E, K]
    # mq[b,q,k] = q[b] @ Vq^T + bias
    mqT = q @ V[:, :D].T + b[None, None, :]              # [B, Q, K]

    nc = _get_compiled()
    in_maps = []
    for c in range(NCORES):
        s = slice(c * BL, (c + 1) * BL)
        mq_shard = np.ascontiguousarray(
            mqT[s].reshape(BL, Q // 128, 128, K).transpose(1, 2, 0, 3)
        )  # [QT, 128, BL, K]
        in_maps.append({
            "qt": np.ascontiguousarray(qt[s]),
            "dat": np.ascontiguousarray(dat[s]),
            "w": w16,
            "vdt": vdt,
            "mq": mq_shard,
        })

    trace = bool(int(os.environ.get("NTN_TRACE", "0"))) and _install_profshim()
    res = bass_utils.run_bass_kernel_spmd(
        nc, in_maps, core_ids=list(range(NCORES)), trace=trace
    )
    if trace and res.exec_time_ns is not None:
        print(f"HW exec time: {res.exec_time_ns} ns")
    out = np.concatenate([r["out"] for r in res.results], axis=0)
    return out


# revision 7
# speedup vs baseline: 1.5059x; 1.0411x over previous
"""Trainium2 Bass kernel for the NTN problem.

out[b,k,q,a] = sigmoid( q[b,q,:] @ w[k] @ da[b,a,:]
                        + Vq[k]@q[b,q,:] + Vd[k]@da[b,a,:] + b[k] )

B=64, K=16, Q=A=D=256.  Sharding: data-parallel over batch B across the
8 NeuronCores (8 batches per core); w/V/b replicated.

Per core, per (k, batch-pair):
  MM1 (TensorE, fp16): tmp[e, q|q'] = sum_d w[k,d,e]^T qT[d, q|q']   (N=512)
  DVE: tmp PSUM->SBUF (fp16) with per-partition bias +Vd[k,e] (folds Vd@da)
  MM2 (TensorE, fp16): out[q, a] = sum_e tmp[e,q]^T daT[e, a]
  ScalarE: sigmoid(psum + bias mq[b,k,q]) where mq = Vq@q + b (host-prepped),
           written into a per-(b,qtile) collect tile covering all 16 k
  One 2 MB DMA per (b, qtile) collect tile -> 16 output stores total.
"""

import os
import sys
import types
from contextlib import ExitStack

if "/opt/trn_rl_repo" not in sys.path:
    sys.path.insert(0, "/opt/trn_rl_repo")

import numpy as np

import concourse.bass as bass
import concourse.tile as tile
from concourse import bacc, bass_utils, mybir

F32 = mybir.dt.float32
F16 = mybir.dt.float16
SIG = mybir.ActivationFunctionType.Sigmoid

NCORES = 8
B, Q, A, D, K = 64, 256, 256, 256, 16
E = D
BL = B // NCORES


def _install_profshim():
    """Provide antenv.axon_hooks so trace=True works under axon (best-effort)."""
    try:
        if "antenv.axon_hooks" in sys.modules:
            return True
        import antenv

        mod = types.ModuleType("antenv.axon_hooks")
        holder = {}
        mod.set_axon_ntff_profile_hook = lambda h: holder.__setitem__("h", h)
        mod.get_axon_ntff_profile_hook = lambda: holder.get("h")
        sys.modules["antenv.axon_hooks"] = mod
        antenv.axon_hooks = mod
        from trn_agent_boot.trn_boot import _ntff_profile_via_ctypes

        hook = _ntff_profile_via_ctypes("/opt/axon/libaxon_pjrt.so")
        if hook is None:
            return False
        mod.set_axon_ntff_profile_hook(hook)
        return True
    except Exception:
        return False


def _build_ntn(tc: tile.TileContext, ctx: ExitStack, aps: dict):
    nc = tc.nc
    DC, ET, QT = D // 128, E // 128, Q // 128
    qt, dat, w, vdt, mq, out = (aps[n] for n in ("qt", "dat", "w", "vdt", "mq", "out"))

    w_pool = ctx.enter_context(tc.tile_pool(name="w", bufs=1))
    const_pool = ctx.enter_context(tc.tile_pool(name="const", bufs=1))
    q_pool = ctx.enter_context(tc.tile_pool(name="q", bufs=3))
    da_pool = ctx.enter_context(tc.tile_pool(name="da", bufs=3))
    tmp_pool = ctx.enter_context(tc.tile_pool(name="tmp", bufs=4))
    out_pool = ctx.enter_context(tc.tile_pool(name="out", bufs=16))
    ptmp_pool = ctx.enter_context(tc.tile_pool(name="ptmp", bufs=3, space="PSUM"))
    pout_pool = ctx.enter_context(tc.tile_pool(name="pout", bufs=5, space="PSUM"))

    # First batch-pair's activations first, so MM1(bp0,k=0) can start early;
    # then per-k w tiles, then the small constants.
    act_tiles = {}

    def load_pair(bp):
        b0, b1 = 2 * bp, 2 * bp + 1
        q2 = q_pool.tile([128, DC, 2 * Q], F16, name=f"q2_{bp}", tag="q2")
        nc.sync.dma_start(q2[:, :, 0:Q], qt[b0].rearrange("(dc p) q -> p dc q", p=128))
        nc.sync.dma_start(q2[:, :, Q:2 * Q], qt[b1].rearrange("(dc p) q -> p dc q", p=128))
        da2 = da_pool.tile([128, ET, 2 * A], F16, name=f"da2_{bp}", tag="da2")
        nc.sync.dma_start(da2[:, :, 0:A], dat[b0].rearrange("(et p) a -> p et a", p=128))
        nc.sync.dma_start(da2[:, :, A:2 * A], dat[b1].rearrange("(et p) a -> p et a", p=128))
        act_tiles[bp] = (q2, da2)

    load_pair(0)
    w_sb = []
    for k in range(K):
        wk = w_pool.tile([128, DC, E], F16, name=f"wk{k}", tag=f"wk{k}")
        nc.sync.dma_start(wk[:], w[k].rearrange("(dc p) e -> p dc e", p=128))
        w_sb.append(wk)
    vdt_sb = const_pool.tile([128, ET, K], F32)
    nc.sync.dma_start(vdt_sb[:], vdt.rearrange("(et p) k -> p et k", p=128))
    mq_sb = const_pool.tile([128, QT, BL, K], F32)
    nc.gpsimd.dma_start(mq_sb[:], mq.rearrange("t p b k -> p t b k"))

    for bp in range(BL // 2):
        b0, b1 = 2 * bp, 2 * bp + 1
        if bp not in act_tiles:
            load_pair(bp)
        q2, da2 = act_tiles.pop(bp)

        # per-(b, qtile, k-half) collect tiles
        KH = K // 4
        coll = {(h, qt_i, kh): out_pool.tile([128, KH, A], F32, name="coll", tag="coll")
                for h in (0, 1) for qt_i in range(QT) for kh in range(K // KH)}

        for k in range(K):
            ptmps = []
            for et in range(ET):
                pt = ptmp_pool.tile([128, 2 * Q], F32)
                for dc in range(DC):
                    nc.tensor.matmul(
                        pt[:],
                        lhsT=w_sb[k][:, dc, et * 128:(et + 1) * 128],
                        rhs=q2[:, dc, :],
                        start=(dc == 0),
                        stop=(dc == DC - 1),
                    )
                ptmps.append(pt)
            tmp = tmp_pool.tile([128, ET, 2 * Q], F16)
            for et in range(ET):
                nc.vector.tensor_scalar_add(
                    tmp[:, et, :], ptmps[et][:], vdt_sb[:, et, k:k + 1]
                )
            for h, b in ((0, b0), (1, b1)):
                for qt_i in range(QT):
                    po = pout_pool.tile([128, A], F32)
                    for et in range(ET):
                        nc.tensor.matmul(
                            po[:],
                            lhsT=tmp[:, et, h * Q + qt_i * 128: h * Q + (qt_i + 1) * 128],
                            rhs=da2[:, et, h * A:(h + 1) * A],
                            start=(et == 0),
                            stop=(et == ET - 1),
                        )
                    nc.scalar.activation(
                        coll[(h, qt_i, k // KH)][:, k % KH, :], po[:], SIG,
                        bias=mq_sb[:, qt_i, b, k:k + 1],
                    )
            if (k + 1) % KH == 0:
                kh = k // KH
                for h, b in ((0, b0), (1, b1)):
                    for qt_i in range(QT):
                        nc.sync.dma_start(
                            out[b, kh * KH:(kh + 1) * KH,
                                qt_i * 128:(qt_i + 1) * 128, :].rearrange("k p a -> p k a"),
                            coll[(h, qt_i, kh)][:],
                        )


_COMPILED = None


def _get_compiled():
    global _COMPILED
    if _COMPILED is not None:
        return _COMPILED
    nc = bacc.Bacc("TRN2", target_bir_lowering=False, debug=False, num_devices=NCORES)
    aps = {
        "qt": nc.dram_tensor("qt", [BL, D, Q], F16, kind="ExternalInput").ap(),
        "dat": nc.dram_tensor("dat", [BL, E, A], F16, kind="ExternalInput").ap(),
        "w": nc.dram_tensor("w", [K, D, E], F16, kind="ExternalInput").ap(),
        "vdt": nc.dram_tensor("vdt", [E, K], F32, kind="ExternalInput").ap(),
        "mq": nc.dram_tensor("mq", [Q // 128, 128, BL, K], F32, kind="ExternalInput").ap(),
        "out": nc.dram_tensor("out", [BL, K, Q, A], F32, kind="ExternalOutput").ap(),
    }
    with tile.TileContext(nc) as tc:
        with ExitStack() as ctx:
            _build_ntn(tc, ctx, aps)
    nc.compile()
    _COMPILED = nc
    return nc


def kernel(batch_q_em, batch_da_em, w, V, b):
    q = np.ascontiguousarray(np.asarray(batch_q_em, dtype=np.float32))
    da = np.ascontiguousarray(np.asarray(batch_da_em, dtype=np.float32))
    w = np.ascontiguousarray(np.asarray(w, dtype=np.float32))
    V = np.ascontiguousarray(np.asarray(V, dtype=np.float32))
    b = np.asarray(b, dtype=np.float32).reshape(-1)

    qt = np.ascontiguousarray(q.transpose(0, 2, 1)).astype(np.float16)   # [B, D, Q]
    dat = np.ascontiguousarray(da.transpose(0, 2, 1)).astype(np.float16)  # [B, E, A]
    w16 = w.astype(np.float16)
    vdt = np.ascontiguousarray(V[:, D:].T)               # [E, K]
    # mq[b,q,k] = q[b] @ Vq^T + bias
    mqT = q @ V[:, :D].T + b[None, None, :]              # [B, Q, K]

    nc = _get_compiled()
    in_maps = []
    for c in range(NCORES):
        s = slice(c * BL, (c + 1) * BL)
        mq_shard = np.ascontiguousarray(
            mqT[s].reshape(BL, Q // 128, 128, K).transpose(1, 2, 0, 3)
        )  # [QT, 128, BL, K]
        in_maps.append({
            "qt": np.ascontiguousarray(qt[s]),
            "dat": np.ascontiguousarray(dat[s]),
            "w": w16,
            "vdt": vdt,
            "mq": mq_shard,
        })

    trace = bool(int(os.environ.get("NTN_TRACE", "0"))) and _install_profshim()
    res = bass_utils.run_bass_kernel_spmd(
        nc, in_maps, core_ids=list(range(NCORES)), trace=trace
    )
    if trace and res.exec_time_ns is not None:
        print(f"HW exec time: {res.exec_time_ns} ns")
    out = np.concatenate([r["out"] for r in res.results], axis=0)
    return out
